# revision 1
# baseline (speedup 1.0000x reference)
"""MatchNet kernel for 8 Trainium2 NeuronCores.

Math (per batch b):
    keys   = q[b] @ W + bias
    scores = p[b] @ keys^T
    attn   = softmax(scores, axis=-1)
    out[b] = relu(attn @ q[b])

Because softmax is shift-invariant along the reduced axis, the Dense bias
contributes p@b^T (constant along lq) to scores and therefore has NO effect
on the output — it is dropped entirely.

Sharding: data-parallel over B=16 across 8 cores (2 batches per core).
W is broadcast. p and q are transposed on the host so every on-chip matmul
has its contraction dim on SBUF partitions.

Precision: the softmax here is extremely sharp (scores std ~32), so plain
bf16/fp32r matmuls in the score path give ~8% output error. Instead each
fp32 operand x of MM1/MM2 is split on the host into a bf16 pair (xb, xr)
with x ≈ xb + xr (~16 mantissa bits combined) and those matmuls run 3 bf16
passes (xb@yb + xb@yr + xr@yb) accumulated in fp32 PSUM at full PE rate:
    MM1: keysT[h, lq] = sum_hk W[hk, h] * qT[hk, lq]   3-pass split
         keysT split on-chip into bf16 pair via DVE copy+sub
    MM2: scores[lp, lq] = sum_h pT[h, lp] * keysT[h, lq]  3-pass split
    softmax over free dim; exp via ACT (bias=-rowmax, accum rowsum),
    exp output stored fp16
    T:   attnT[lq, lp] via PE transpose (fp16)
    MM3: out[lp, h] = sum_lq attnT[lq, lp] * q[lq, h]  single fp16 pass
    relu(out * (1/rowsum)) via ACT with per-partition scale
MM3 tolerates fp16 (attn in [0,1], result not softmax-amplified).
Measured on HW: 423 us/core exec, max |err| / max |out| = 3.75e-4.
(DMA xbar transpose for attnT was tried instead of PE transposes and was
~110us SLOWER end-to-end — DMATranspose<->DMACopy xbar-mode serialization
stalls the regular DMA streams. Keep the PE-transpose path.)
"""

import os
from contextlib import ExitStack

import ml_dtypes
import numpy as np

import concourse.bass as bass
import concourse.mybir as mybir
import concourse.tile as tile
from concourse import bacc
from concourse.bass import ts
from concourse.bass_utils import run_bass_kernel_spmd
from concourse.masks import make_identity

B, L, H = 16, 1024, 1024
NCORES = 8
BPC = B // NCORES  # batches per core
P = 128
KO = H // P        # 8 contraction chunks
NT = L // P        # 8 lp tiles per batch
NF = 512           # matmul moving free dim
NCH = L // NF      # 2 free chunks
F32 = mybir.dt.float32
BF16 = mybir.dt.bfloat16
FP16 = mybir.dt.float16
AF = mybir.ActivationFunctionType
AX = mybir.AxisListType


def _build_body(ctx, tc, ins, out):
    nc = tc.nc
    pTb, pTr, qTb, qTr, qn16, Wb, Wr = ins

    # PE warmup: the first ~17us are DMA-bound (bootstrap + first loads) and
    # the PE would sit idle, entering the kernel HAM-throttled at 1.2 GHz.
    # ~60 zero matmuls during that window cost nothing and flip the clock
    # gate to 2.4 GHz before the real matmuls start.
    with (
        tc.tile_pool(name="warm", bufs=1) as warm_pool,
        tc.tile_pool(name="warmps", bufs=1, space=bass.MemorySpace.PSUM) as wps_pool,
    ):
        wsb = warm_pool.tile([P, P], BF16)
        nc.gpsimd.memset(wsb[:], 0.0)
        wps = wps_pool.tile([P, P], F32)
        for _ in range(60):
            nc.tensor.matmul(wps[:], wsb[:], wsb[:], start=True, stop=True)

    const = ctx.enter_context(tc.tile_pool(name="const", bufs=1))
    # W splits, one tile per k-chunk (chunk-granular deps let the first
    # matmul start as soon as chunk 0 lands instead of after the full 4MB).
    # Wb issued first: MM1 pass 1 only needs (Wb, qTb), so the PE can start
    # while qTr/Wr/qn are still loading.
    Wb_sb = [const.tile([P, H], BF16, name=f"Wb_sb_{k}") for k in range(KO)]
    Wr_sb = [const.tile([P, H], BF16, name=f"Wr_sb_{k}") for k in range(KO)]
    ident = const.tile([P, P], FP16)
    make_identity(nc, ident[:])

    qT_pool = ctx.enter_context(tc.tile_pool(name="qTp", bufs=1))
    q_pool = ctx.enter_context(tc.tile_pool(name="qp", bufs=1))
    keysT_pool = ctx.enter_context(tc.tile_pool(name="keysTp", bufs=1))
    pT_pool = ctx.enter_context(tc.tile_pool(name="pTp", bufs=3))
    attn_pool = ctx.enter_context(tc.tile_pool(name="attnp", bufs=2))
    attnT_pool = ctx.enter_context(tc.tile_pool(name="attnTp", bufs=2))
    osb_pool = ctx.enter_context(tc.tile_pool(name="osbp", bufs=2))
    stat_pool = ctx.enter_context(tc.tile_pool(name="statp", bufs=8))
    ps_big = ctx.enter_context(
        tc.tile_pool(name="psbig", bufs=3, space=bass.MemorySpace.PSUM)
    )
    ps_t = ctx.enter_context(
        tc.tile_pool(name="pst", bufs=2, space=bass.MemorySpace.PSUM)
    )

    Wb_re = Wb.rearrange("(ko ki) h -> ki ko h", ki=P)
    Wr_re = Wr.rearrange("(ko ki) h -> ki ko h", ki=P)

    for b in range(BPC):
        # qT splits first (MM1 needs them); base before residual.
        # Interleave Wb/qTb chunk loads so the pair needed by each MM1
        # k-step lands on adjacent DMA queues (both early), not queued
        # behind each other.
        qTb_sb = [
            qT_pool.tile([P, L], BF16, name=f"qTb_sb_{b}_{k}", tag=f"qTb_sb{k}")
            for k in range(KO)
        ]
        qTr_sb = [
            qT_pool.tile([P, L], BF16, name=f"qTr_sb_{b}_{k}", tag=f"qTr_sb{k}")
            for k in range(KO)
        ]
        qTb_re = qTb[b].rearrange("(ko ki) l -> ki ko l", ki=P)
        qTr_re = qTr[b].rearrange("(ko ki) l -> ki ko l", ki=P)
        for k in range(KO):
            if b == 0:
                nc.sync.dma_start(Wb_sb[k][:], Wb_re[:, k, :])
            nc.sync.dma_start(qTb_sb[k][:], qTb_re[:, k, :])
        for k in range(KO):
            nc.sync.dma_start(qTr_sb[k][:], qTr_re[:, k, :])
        if b == 0:
            for k in range(KO):
                nc.sync.dma_start(Wr_sb[k][:], Wr_re[:, k, :])

        # ---- phase 1: keysT[h, lq] = (q @ W)^T, 3-pass split, then split
        kb_sb = keysT_pool.tile([P, KO, L], BF16, name=f"kb_{b}", tag="kb")
        kr_sb = keysT_pool.tile([P, KO, L], BF16, name=f"kr_{b}", tag="kr")
        mm1_pairs = ((Wb_sb, qTb_sb), (Wb_sb, qTr_sb), (Wr_sb, qTb_sb))
        for m in range(KO):
            ps_k = ps_big.tile([P, L], F32, name=f"ps_k_{b}_{m}", tag="ps_big")
            for n in range(NCH):
                for pi, (Asb, Bsb) in enumerate(mm1_pairs):
                    for k in range(KO):
                        nc.tensor.matmul(
                            ps_k[:, ts(n, NF)],
                            Asb[k][:, ts(m, P)],
                            Bsb[k][:, ts(n, NF)],
                            start=(pi == 0 and k == 0),
                            stop=(pi == len(mm1_pairs) - 1 and k == KO - 1),
                        )
            nc.vector.tensor_copy(kb_sb[:, m, :], ps_k[:])
            nc.vector.tensor_sub(kr_sb[:, m, :], ps_k[:], kb_sb[:, m, :])

        # q natural (fp16, for MM3): issued after phase-1 compute so its DMA
        # queues drain behind the phase-1-critical loads.
        qn_sb = q_pool.tile([P, KO, H], FP16, name=f"qn_sb_{b}", tag="qn_sb")
        qre = qn16[b].rearrange("(ko ki) h -> ki ko h", ki=P)
        for k in range(KO):
            nc.sync.dma_start(qn_sb[:, k, :], qre[:, k, :])

        # ---- phase 2/3: per lp tile, software-pipelined
        pTb_r = pTb[b].rearrange("(ko ki) l -> ki ko l", ki=P)
        pTr_r = pTr[b].rearrange("(ko ki) l -> ki ko l", ki=P)
        scores_ps = {}
        soft = {}

        def stage_scores(i, b=b, pTb_r=pTb_r, pTr_r=pTr_r, kb_sb=kb_sb, kr_sb=kr_sb):
            pb_sb = pT_pool.tile([P, KO, P], BF16, name=f"pb_sb_{b}_{i}", tag="pb_sb")
            pr_sb = pT_pool.tile([P, KO, P], BF16, name=f"pr_sb_{b}_{i}", tag="pr_sb")
            nc.sync.dma_start(pb_sb[:], pTb_r[:, :, ts(i, P)])
            nc.sync.dma_start(pr_sb[:], pTr_r[:, :, ts(i, P)])
            ps_s = ps_big.tile([P, L], F32, name=f"ps_s_{b}_{i}", tag="ps_big")
            mm2_pairs = ((pb_sb, kb_sb), (pb_sb, kr_sb), (pr_sb, kb_sb))
            for n in range(NCH):
                for pi, (Asb, Bsb) in enumerate(mm2_pairs):
                    for k in range(KO):
                        nc.tensor.matmul(
                            ps_s[:, ts(n, NF)],
                            Asb[:, k, :],
                            Bsb[:, k, ts(n, NF)],
                            start=(pi == 0 and k == 0),
                            stop=(pi == len(mm2_pairs) - 1 and k == KO - 1),
                        )
            scores_ps[i] = ps_s

        def stage_softmax_t(i, b=b):
            ps_s = scores_ps.pop(i)
            negmax = stat_pool.tile([P, 1], F32, name=f"negmax_{b}_{i}", tag="negmax")
            nc.vector.reduce_max(negmax[:], ps_s[:], axis=AX.X, negate=True)
            attn_sb = attn_pool.tile([P, L], FP16, name=f"attn_{b}_{i}", tag="attn")
            rowsum = stat_pool.tile([P, 1], F32, name=f"rowsum_{b}_{i}", tag="rowsum")
            nc.scalar.activation(
                attn_sb[:],
                ps_s[:],
                AF.Exp,
                bias=negmax[:],
                accum_out=rowsum[:],
            )
            recip = stat_pool.tile([P, 1], F32, name=f"recip_{b}_{i}", tag="recip")
            nc.vector.reciprocal(recip[:], rowsum[:])

            attnT_sb = attnT_pool.tile([P, L], FP16, name=f"attnT_{b}_{i}", tag="attnT")
            for g in range(L // NF):
                ps_tt = ps_t.tile([P, NF], FP16, name=f"ps_tt_{b}_{i}_{g}", tag="ps_t")
                for j in range(NF // P):
                    c = g * (NF // P) + j
                    nc.tensor.transpose(
                        ps_tt[:, ts(j, P)], attn_sb[:, ts(c, P)], ident[:]
                    )
                nc.vector.tensor_copy(attnT_sb[:, ts(g, NF)], ps_tt[:])
            soft[i] = (attnT_sb, recip)

        def stage_mm3(i, b=b, qn_sb=qn_sb):
            attnT_sb, recip = soft.pop(i)
            out_sb = osb_pool.tile([P, H], F32, name=f"out_sb_{b}_{i}", tag="out_sb")
            ps_o = ps_big.tile([P, H], F32, name=f"ps_o_{b}_{i}", tag="ps_big")
            # relu+store per n-chunk so the drain of chunk 0 hides under the
            # matmuls of chunk 1 (shrinks the kernel tail).
            for n in range(NCH):
                for k in range(KO):
                    nc.tensor.matmul(
                        ps_o[:, ts(n, NF)],
                        attnT_sb[:, ts(k, P)],
                        qn_sb[:, k, ts(n, NF)],
                        start=(k == 0),
                        stop=(k == KO - 1),
                    )
                nc.scalar.activation(
                    out_sb[:, ts(n, NF)], ps_o[:, ts(n, NF)], AF.Relu, scale=recip[:]
                )
                nc.sync.dma_start(out[b, ts(i, P), ts(n, NF)], out_sb[:, ts(n, NF)])

        stage_scores(0)
        stage_scores(1)
        for i in range(NT):
            stage_softmax_t(i)
            if i + 2 < NT:
                stage_scores(i + 2)
            stage_mm3(i)


_IN_NAMES = ["pTb", "pTr", "qTb", "qTr", "qn16", "Wb", "Wr"]

_CACHED = None


def _get_program():
    global _CACHED
    if _CACHED is not None:
        return _CACHED
    nc = bacc.Bacc(
        "TRN2",
        target_bir_lowering=False,
        debug=False,
        num_devices=NCORES,
    )
    specs = {
        "pTb": ([BPC, H, L], BF16),
        "pTr": ([BPC, H, L], BF16),
        "qTb": ([BPC, H, L], BF16),
        "qTr": ([BPC, H, L], BF16),
        "qn16": ([BPC, L, H], FP16),
        "Wb": ([H, H], BF16),
        "Wr": ([H, H], BF16),
    }
    handles = [
        nc.dram_tensor(name, *specs[name], kind="ExternalInput") for name in _IN_NAMES
    ]
    out_h = nc.dram_tensor("out", [BPC, L, H], F32, kind="ExternalOutput")
    with tile.TileContext(nc) as tc:
        with ExitStack() as ctx:
            _build_body(ctx, tc, [h.ap() for h in handles], out_h.ap())
    nc.compile()
    _CACHED = nc
    return nc


def _split_bf16(x):
    xb = x.astype(ml_dtypes.bfloat16)
    xr = (x - xb.astype(np.float32)).astype(ml_dtypes.bfloat16)
    return xb, xr


def kernel(p, q, W_key, b_key):
    # b_key is mathematically irrelevant: softmax over lq is invariant to the
    # per-lp constant p@b^T it adds to scores, and keys are not used elsewhere.
    del b_key
    p = np.ascontiguousarray(np.asarray(p, dtype=np.float32))
    q = np.ascontiguousarray(np.asarray(q, dtype=np.float32))
    W = np.ascontiguousarray(np.asarray(W_key, dtype=np.float32))
    pT = np.ascontiguousarray(p.transpose(0, 2, 1))
    qT = np.ascontiguousarray(q.transpose(0, 2, 1))

    pTb, pTr = _split_bf16(pT)
    qTb, qTr = _split_bf16(qT)
    qn16 = q.astype(np.float16)
    Wb, Wr = _split_bf16(W)
    full = {
        "pTb": pTb, "pTr": pTr,
        "qTb": qTb, "qTr": qTr,
        "qn16": qn16,
    }

    in_maps = []
    for c in range(NCORES):
        sl = slice(c * BPC, (c + 1) * BPC)
        m = {k: np.ascontiguousarray(v[sl]) for k, v in full.items()}
        m["Wb"] = Wb
        m["Wr"] = Wr
        in_maps.append(m)

    nc = _get_program()
    trace = bool(int(os.environ.get("MATCHNET_TRACE", "0")))
    res = run_bass_kernel_spmd(nc, in_maps, list(range(NCORES)), trace=trace)
    if trace:
        kernel.last_exec_time_ns = res.exec_time_ns
        kernel.last_results = res
    out = np.concatenate([res.results[c]["out"] for c in range(NCORES)], axis=0)
    return out


kernel.last_exec_time_ns = None
kernel.last_results = None



# revision 3
# speedup vs baseline: 1.3607x; 1.3607x over previous
"""MatchNet kernel for 8 Trainium2 NeuronCores.

Math (per batch b):
    keys   = q[b] @ W + bias
    scores = p[b] @ keys^T
    attn   = softmax(scores, axis=-1)
    out[b] = relu(attn @ q[b])

Because softmax is shift-invariant along the reduced axis, the Dense bias
contributes p@b^T (constant along lq) to scores and therefore has NO effect
on the output — it is dropped entirely.

Sharding: data-parallel over B=16 across 8 cores (2 batches per core).
W is broadcast. p and q are transposed on the host so every on-chip matmul
has its contraction dim on SBUF partitions.

Score path (4 matmul passes instead of the previous 6): associativity is
used to fold W onto p instead of q:
    scores = p @ (qW)^T = (p @ W^T) @ q^T = G @ q^T
and each matmul runs asymmetric precision — a 2-term high-precision pair on
one side times a single fp16 operand on the other (PE takes mixed
bf16/fp16 operands; products are exact in fp32 PSUM):
    MM1: GT[h, lp] = sum_hk (Wb+Wr)[hk, h] * pT16[hk, lp]   2 bf16xfp16 passes
         (Wb, Wr) = bf16 split of W (host), pT16 = fp16(p^T) (host)
         GT split on-chip into bf16 pair (GTb, GTr) via DVE copy+sub
    MM2: scores[lp, lq] = sum_h (GTb+GTr)[h, lp] * qT16[h, lq]  2 passes
    softmax over free dim; exp via ACT (bias=-rowmax, accum rowsum),
    exp output stored fp16
    T:   attnT[lq, lp] via PE transpose (fp16)
    MM3: out[lp, h] = sum_lq attnT[lq, lp] * qn16[lq, h]  single fp16 pass
    relu(out * (1/rowsum)) via ACT with per-partition scale
Error budget: each single-fp16 side contributes ~7e-3 of max-rel error
through the sharp softmax; numpy simulation of this exact scheme on the
harness inputs gives rel 9.36e-3 vs the 2e-2 gate (the same simulator
reproduces the old 6-pass kernel's HW-measured 3.75e-4 exactly).
(DMA xbar transpose for attnT was tried instead of PE transposes and was
~110us SLOWER end-to-end — DMATranspose<->DMACopy xbar-mode serialization
stalls the regular DMA streams. Keep the PE-transpose path.)
"""

import os
from contextlib import ExitStack

import ml_dtypes
import numpy as np

import concourse.bass as bass
import concourse.mybir as mybir
import concourse.tile as tile
from concourse import bacc
from concourse.bass import ts
from concourse.bass_utils import run_bass_kernel_spmd
from concourse.masks import make_identity

B, L, H = 16, 1024, 1024
NCORES = 8
BPC = B // NCORES  # batches per core
P = 128
KO = H // P        # 8 contraction chunks
NT = L // P        # 8 lp tiles per batch
NF = 512           # matmul moving free dim
NCH = L // NF      # 2 free chunks
F32 = mybir.dt.float32
BF16 = mybir.dt.bfloat16
FP16 = mybir.dt.float16
AF = mybir.ActivationFunctionType
AX = mybir.AxisListType


def _build_body(ctx, tc, ins, out):
    nc = tc.nc
    pT16, qT16, qn16, Wb, Wr = ins

    # PE warmup: the first ~15us are DMA-bound (bootstrap + first loads) and
    # the PE would sit idle, entering the kernel HAM-throttled at 1.2 GHz.
    # Zero matmuls during that window cost nothing and flip the clock
    # gate to 2.4 GHz before the real matmuls start.
    with (
        tc.tile_pool(name="warm", bufs=1) as warm_pool,
        tc.tile_pool(name="warmps", bufs=1, space=bass.MemorySpace.PSUM) as wps_pool,
    ):
        wsb = warm_pool.tile([P, P], BF16)
        nc.gpsimd.memset(wsb[:], 0.0)
        wps = wps_pool.tile([P, P], F32)
        for _ in range(60):
            nc.tensor.matmul(wps[:], wsb[:], wsb[:], start=True, stop=True)

    const = ctx.enter_context(tc.tile_pool(name="const", bufs=1))
    # W splits, one tile per k-chunk (chunk-granular deps let the first
    # matmul start as soon as chunk 0 lands instead of after the full 4MB).
    # Wb issued first: MM1 pass 1 only needs (Wb, pT16), so the PE can start
    # while Wr is still loading.
    Wb_sb = [const.tile([P, H], BF16, name=f"Wb_sb_{k}") for k in range(KO)]
    Wr_sb = [const.tile([P, H], BF16, name=f"Wr_sb_{k}") for k in range(KO)]
    ident = const.tile([P, P], FP16)
    make_identity(nc, ident[:])

    pT_pool = ctx.enter_context(tc.tile_pool(name="pTp", bufs=2))
    qT_pool = ctx.enter_context(tc.tile_pool(name="qTp", bufs=2))
    q_pool = ctx.enter_context(tc.tile_pool(name="qp", bufs=2))
    gT_pool = ctx.enter_context(tc.tile_pool(name="gTp", bufs=1))
    attn_pool = ctx.enter_context(tc.tile_pool(name="attnp", bufs=2))
    attnT_pool = ctx.enter_context(tc.tile_pool(name="attnTp", bufs=2))
    osb_pool = ctx.enter_context(tc.tile_pool(name="osbp", bufs=2))
    stat_pool = ctx.enter_context(tc.tile_pool(name="statp", bufs=8))
    ps_big = ctx.enter_context(
        tc.tile_pool(name="psbig", bufs=3, space=bass.MemorySpace.PSUM)
    )
    ps_t = ctx.enter_context(
        tc.tile_pool(name="pst", bufs=2, space=bass.MemorySpace.PSUM)
    )

    Wb_re = Wb.rearrange("(ko ki) h -> ki ko h", ki=P)
    Wr_re = Wr.rearrange("(ko ki) h -> ki ko h", ki=P)

    for b in range(BPC):
        # pT16 chunks first (MM1-critical), interleaved with Wb for b==0 so
        # the pair needed by each MM1 k-step lands on adjacent DMA queues.
        pT_sb = [
            pT_pool.tile([P, L], FP16, name=f"pT_sb_{b}_{k}", tag=f"pT_sb{k}")
            for k in range(KO)
        ]
        qT_sb = [
            qT_pool.tile([P, L], FP16, name=f"qT_sb_{b}_{k}", tag=f"qT_sb{k}")
            for k in range(KO)
        ]
        pT_re = pT16[b].rearrange("(ko ki) l -> ki ko l", ki=P)
        qT_re = qT16[b].rearrange("(ko ki) l -> ki ko l", ki=P)
        for k in range(KO):
            if b == 0:
                nc.sync.dma_start(Wb_sb[k][:], Wb_re[:, k, :])
            nc.sync.dma_start(pT_sb[k][:], pT_re[:, k, :])
        if b == 0:
            for k in range(KO):
                nc.sync.dma_start(Wr_sb[k][:], Wr_re[:, k, :])
        for k in range(KO):
            nc.sync.dma_start(qT_sb[k][:], qT_re[:, k, :])

        # ---- phase 1: GT[h, lp] = (p @ W^T)^T, 2-pass asym, then bf16 split
        gb_sb = gT_pool.tile([P, KO, L], BF16, name=f"gb_{b}", tag="gb")
        gr_sb = gT_pool.tile([P, KO, L], BF16, name=f"gr_{b}", tag="gr")
        mm1_passes = (Wb_sb, Wr_sb)
        for m in range(KO):
            ps_k = ps_big.tile([P, L], F32, name=f"ps_k_{b}_{m}", tag="ps_big")
            for n in range(NCH):
                for pi, Asb in enumerate(mm1_passes):
                    for k in range(KO):
                        nc.tensor.matmul(
                            ps_k[:, ts(n, NF)],
                            Asb[k][:, ts(m, P)],
                            pT_sb[k][:, ts(n, NF)],
                            start=(pi == 0 and k == 0),
                            stop=(pi == len(mm1_passes) - 1 and k == KO - 1),
                        )
            nc.vector.tensor_copy(gb_sb[:, m, :], ps_k[:])
            nc.vector.tensor_sub(gr_sb[:, m, :], ps_k[:], gb_sb[:, m, :])

        # q natural (fp16, for MM3): issued after phase-1 compute so its DMA
        # queues drain behind the phase-1-critical loads.
        qn_sb = q_pool.tile([P, KO, H], FP16, name=f"qn_sb_{b}", tag="qn_sb")
        qre = qn16[b].rearrange("(ko ki) h -> ki ko h", ki=P)
        for k in range(KO):
            nc.sync.dma_start(qn_sb[:, k, :], qre[:, k, :])

        # ---- phase 2/3: per lp tile, software-pipelined
        scores_ps = {}
        soft = {}

        def stage_scores(i, b=b, gb_sb=gb_sb, gr_sb=gr_sb, qT_sb=qT_sb):
            ps_s = ps_big.tile([P, L], F32, name=f"ps_s_{b}_{i}", tag="ps_big")
            mm2_passes = (gb_sb, gr_sb)
            for n in range(NCH):
                for pi, Gsb in enumerate(mm2_passes):
                    for k in range(KO):
                        nc.tensor.matmul(
                            ps_s[:, ts(n, NF)],
                            Gsb[:, k, ts(i, P)],
                            qT_sb[k][:, ts(n, NF)],
                            start=(pi == 0 and k == 0),
                            stop=(pi == len(mm2_passes) - 1 and k == KO - 1),
                        )
            scores_ps[i] = ps_s

        def stage_softmax_t(i, b=b):
            ps_s = scores_ps.pop(i)
            negmax = stat_pool.tile([P, 1], F32, name=f"negmax_{b}_{i}", tag="negmax")
            nc.vector.reduce_max(negmax[:], ps_s[:], axis=AX.X, negate=True)
            attn_sb = attn_pool.tile([P, L], FP16, name=f"attn_{b}_{i}", tag="attn")
            rowsum = stat_pool.tile([P, 1], F32, name=f"rowsum_{b}_{i}", tag="rowsum")
            nc.scalar.activation(
                attn_sb[:],
                ps_s[:],
                AF.Exp,
                bias=negmax[:],
                accum_out=rowsum[:],
            )
            recip = stat_pool.tile([P, 1], F32, name=f"recip_{b}_{i}", tag="recip")
            nc.vector.reciprocal(recip[:], rowsum[:])

            attnT_sb = attnT_pool.tile([P, L], FP16, name=f"attnT_{b}_{i}", tag="attnT")
            for g in range(L // NF):
                ps_tt = ps_t.tile([P, NF], FP16, name=f"ps_tt_{b}_{i}_{g}", tag="ps_t")
                for j in range(NF // P):
                    c = g * (NF // P) + j
                    nc.tensor.transpose(
                        ps_tt[:, ts(j, P)], attn_sb[:, ts(c, P)], ident[:]
                    )
                nc.vector.tensor_copy(attnT_sb[:, ts(g, NF)], ps_tt[:])
            soft[i] = (attnT_sb, recip)

        def stage_mm3(i, b=b, qn_sb=qn_sb):
            attnT_sb, recip = soft.pop(i)
            out_sb = osb_pool.tile([P, H], F32, name=f"out_sb_{b}_{i}", tag="out_sb")
            ps_o = ps_big.tile([P, H], F32, name=f"ps_o_{b}_{i}", tag="ps_big")
            # relu+store per n-chunk so the drain of chunk 0 hides under the
            # matmuls of chunk 1 (shrinks the kernel tail).
            for n in range(NCH):
                for k in range(KO):
                    nc.tensor.matmul(
                        ps_o[:, ts(n, NF)],
                        attnT_sb[:, ts(k, P)],
                        qn_sb[:, k, ts(n, NF)],
                        start=(k == 0),
                        stop=(k == KO - 1),
                    )
                nc.scalar.activation(
                    out_sb[:, ts(n, NF)], ps_o[:, ts(n, NF)], AF.Relu, scale=recip[:]
                )
                nc.sync.dma_start(out[b, ts(i, P), ts(n, NF)], out_sb[:, ts(n, NF)])

        stage_scores(0)
        stage_scores(1)
        for i in range(NT):
            stage_softmax_t(i)
            if i + 2 < NT:
                stage_scores(i + 2)
            stage_mm3(i)


_IN_NAMES = ["pT16", "qT16", "qn16", "Wb", "Wr"]

_CACHED = None


def _get_program():
    global _CACHED
    if _CACHED is not None:
        return _CACHED
    nc = bacc.Bacc(
        "TRN2",
        target_bir_lowering=False,
        debug=False,
        num_devices=NCORES,
    )
    specs = {
        "pT16": ([BPC, H, L], FP16),
        "qT16": ([BPC, H, L], FP16),
        "qn16": ([BPC, L, H], FP16),
        "Wb": ([H, H], BF16),
        "Wr": ([H, H], BF16),
    }
    handles = [
        nc.dram_tensor(name, *specs[name], kind="ExternalInput") for name in _IN_NAMES
    ]
    out_h = nc.dram_tensor("out", [BPC, L, H], F32, kind="ExternalOutput")
    with tile.TileContext(nc) as tc:
        with ExitStack() as ctx:
            _build_body(ctx, tc, [h.ap() for h in handles], out_h.ap())
    nc.compile()
    _CACHED = nc
    return nc


def _split_bf16(x):
    xb = x.astype(ml_dtypes.bfloat16)
    xr = (x - xb.astype(np.float32)).astype(ml_dtypes.bfloat16)
    return xb, xr


def kernel(p, q, W_key, b_key):
    # b_key is mathematically irrelevant: softmax over lq is invariant to the
    # per-lp constant p@b^T it adds to scores, and keys are not used elsewhere.
    del b_key
    p = np.ascontiguousarray(np.asarray(p, dtype=np.float32))
    q = np.ascontiguousarray(np.asarray(q, dtype=np.float32))
    W = np.ascontiguousarray(np.asarray(W_key, dtype=np.float32))

    pT16 = np.ascontiguousarray(p.transpose(0, 2, 1)).astype(np.float16)
    qT16 = np.ascontiguousarray(q.transpose(0, 2, 1)).astype(np.float16)
    qn16 = q.astype(np.float16)
    # MM1 contracts over W's OUTPUT dim (G = p @ W^T), so the stationary
    # operand layout is [h_out (contraction), h_in] = W transposed.
    Wb, Wr = _split_bf16(np.ascontiguousarray(W.T))
    full = {"pT16": pT16, "qT16": qT16, "qn16": qn16}

    in_maps = []
    for c in range(NCORES):
        sl = slice(c * BPC, (c + 1) * BPC)
        m = {k: np.ascontiguousarray(v[sl]) for k, v in full.items()}
        m["Wb"] = Wb
        m["Wr"] = Wr
        in_maps.append(m)

    nc = _get_program()
    trace = bool(int(os.environ.get("MATCHNET_TRACE", "0")))
    res = run_bass_kernel_spmd(nc, in_maps, list(range(NCORES)), trace=trace)
    if trace:
        kernel.last_exec_time_ns = res.exec_time_ns
        kernel.last_results = res
    out = np.concatenate([res.results[c]["out"] for c in range(NCORES)], axis=0)
    return out


kernel.last_exec_time_ns = None
kernel.last_results = None
